# revision 1
# baseline (speedup 1.0000x reference)
"""Euclidean distance layer on 8 Trainium2 NeuronCores.

out[b, o] = || x[b, :] - weight[:, o] ||_2
x: [512, 256] f32, weight: [256, 1024] f32 -> out: [512, 1024] f32

Sharding: tensor-parallel over output features (8 x 128 columns per core).

Per core:  dist^2 = -2 * ( x~@w~_loc - 0.5*||w~_loc||^2 ) + ||x||^2
where x~, w~ are fp16 roundings of x, w (the x.w and ||w||^2 terms tolerate
fp16 easily; ||x||^2 stays fp32 -> ~2e-5 relative error on dist).
  - x~@w~ on the PE into one fp32 PSUM bank [128, 4x128]
  - ||w~||^2: fp16 squares (DVE) -> [-0.25] x2-column reduce matmul ->
    stride-0 broadcast row copy -> K=2 fp16 fold matmuls
  - ||x||^2 per-partition fp32 columns, split: batch tiles 0/1 via ACT
    Square+accum_out, tiles 2/3 via DVE mul+reduce
  - final: out = sqrt(-2 * psum + ||x||^2) on ACT (affine scale + bias)
Raw bacc, manual semaphores. Input DMAs issue in the pre-block preamble.
Host work is layout/dtype prep only.
"""

from contextlib import ExitStack

import numpy as np

B = 512      # batch
K = 256      # inputSize (contraction dim)
NOUT = 1024  # outputSize
NCORES = 8
NLOC = NOUT // NCORES  # 128 output features per core
P = 128                # partitions
KT = K // P            # 2 contraction chunks
MT = B // P            # 4 batch tiles

_NC = None  # cached compiled Bass program (same SPMD program on all cores)


def _build():
    import concourse.bass as bass
    from concourse import bacc, mybir

    f32 = mybir.dt.float32
    f16 = mybir.dt.float16
    Sqrt = mybir.ActivationFunctionType.Sqrt
    Square = mybir.ActivationFunctionType.Square
    ts = bass.ts

    nc = bacc.Bacc(
        "TRN2", target_bir_lowering=False, debug=False, num_devices=NCORES
    )

    xtf = nc.dram_tensor("xtf", [K, B], f16, kind="ExternalInput")
    xn = nc.dram_tensor("xn", [B, K], f32, kind="ExternalInput")
    wlf = nc.dram_tensor("wlf", [K, NLOC], f16, kind="ExternalInput")
    out = nc.dram_tensor("out", [B, NLOC], f32, kind="ExternalOutput")

    with ExitStack() as ctx:
        e = ctx.enter_context
        xtf_sb = e(nc.sbuf_tensor("xtfs", [P, KT, B], f16))
        wlf_sb = e(nc.sbuf_tensor("wlfs", [P, KT, NLOC], f16))
        xn_sb = [e(nc.sbuf_tensor(f"xns{h}", [P, 2, K], f32)) for h in range(2)]
        wlsq = [e(nc.sbuf_tensor(f"wlsq{k}", [P, NLOC], f16)) for k in range(KT)]
        xsq_scrA = e(nc.sbuf_tensor("xsqsA", [P, 2, K], f32))
        xsq_scrD = e(nc.sbuf_tensor("xsqsD", [P, 2, K], f32))
        xsq_colA = e(nc.sbuf_tensor("xsqcA", [P, 2], f32))
        xsq_colD = e(nc.sbuf_tensor("xsqcD", [P, 2], f32))
        neg_q = e(nc.sbuf_tensor("neg_q", [P, 2], f16))
        ones_m = e(nc.sbuf_tensor("ones_m", [2, P], f16))
        wsq_row4 = e(nc.sbuf_tensor("wsq_row4", [2, MT, NLOC], f16))
        out_sb = e(nc.sbuf_tensor("out_sb", [P, MT, NLOC], f32))
        actwarm = e(nc.sbuf_tensor("actwarm", [1, 1], f32))

        ps_w = e(nc.psum_tensor("ps_w", [2, NLOC], f32))   # -0.25*||w||^2 x2
        ps_all = e(nc.psum_tensor("ps_all", [P, MT, NLOC], f32))  # one bank

        s_wl = e(nc.semaphore("s_wl"))
        s_xt = e(nc.semaphore("s_xt"))
        s_xn = [e(nc.semaphore(f"s_xn{h}")) for h in range(2)]
        s_sq = e(nc.semaphore("s_sq"))      # 2 = both wlsq done
        s_mm = e(nc.semaphore("s_mm"))      # 1 = wsq reduce, 2+m = aug m
        s_brd = e(nc.semaphore("s_brd"))    # 1 = wsq_row4 broadcast ready
        s_colD = e(nc.semaphore("s_colD"))  # 1 = xsq cols for m2/m3 ready
        s_sqrt = e(nc.semaphore("s_sqrt"))  # m+1 = sqrt tile m in out_sb
        s_out = e(nc.semaphore("s_out"))    # 16 = sync output DMA landed
        s_out2 = e(nc.semaphore("s_out2"))  # 16 = scalar output DMA landed

        block = e(nc.Block())

        @block.sync
        def _(sync):
            sync.dma_start(
                out=xn_sb[0][:, :, :],
                in_=xn[0 : 2 * P, :].rearrange("(c p) k -> p c k", p=P),
            ).then_inc(s_xn[0], 16)
            sync.dma_start(
                out=wlf_sb[:, :, :],
                in_=wlf[:, :].rearrange("(c p) o -> p c o", p=P),
            ).then_inc(s_wl, 16)
            sync.dma_start(
                out=xtf_sb[:, :, :],
                in_=xtf[:, :].rearrange("(c p) b -> p c b", p=P),
            ).then_inc(s_xt, 16)
            sync.wait_ge(s_sqrt, 2)
            sync.dma_start(
                out=out[0 : 2 * P, :].rearrange("(m p) o -> p m o", p=P),
                in_=out_sb[:, 0:2, :],
            ).then_inc(s_out, 16)
            sync.wait_ge(s_out, 16)

        @block.gpsimd
        def _(gpsimd):
            gpsimd.dma_start(
                out=xn_sb[1][:, :, :],
                in_=xn[2 * P : 4 * P, :].rearrange("(c p) k -> p c k", p=P),
            ).then_inc(s_xn[1], 16)
            gpsimd.wait_ge(s_xn[1], 16)

        @block.scalar
        def _(scalar):
            # ||x||^2 for batch tiles 0/1 (fp32, Square + free-dim accum)
            # NOTE: also gated on s_xn[1] -- ACT accum activations racing
            # in-flight DMA traffic crash the exec unit on this stack
            scalar.wait_ge(s_xn[1], 16)
            scalar.wait_ge(s_xn[0], 16)
            for m in range(2):
                scalar.activation(
                    xsq_scrA[:, m, :], xn_sb[0][:, m, :], Square,
                    accum_out=xsq_colA[:, m : m + 1],
                )
            scalar.drain()  # ACT RAW: sqrts below read xsq_colA
            for m in range(MT):
                scalar.wait_ge(s_mm, 2 + m)
                if m == 2:
                    scalar.wait_ge(s_colD, 1)
                bias = (
                    xsq_colA[:, m : m + 1] if m < 2
                    else xsq_colD[:, m - 2 : m - 1]
                )
                scalar.activation(
                    out_sb[:, m, :], ps_all[:, m, :], Sqrt,
                    bias=bias, scale=-2.0,
                ).then_inc(s_sqrt)
            scalar.wait_ge(s_sqrt, MT)
            scalar.dma_start(
                out=out[2 * P : 4 * P, :].rearrange("(m p) o -> p m o", p=P),
                in_=out_sb[:, 2:4, :],
            ).then_inc(s_out2, 16)
            scalar.wait_ge(s_out2, 16)


        @block.vector
        def _(vector):
            vector.memset(neg_q[:, :], -0.25)
            vector.memset(ones_m[:, :], 1.0)
            vector.wait_ge(s_wl, 16)
            vector.tensor_mul(wlsq[0][:, :], wlf_sb[:, 0, :], wlf_sb[:, 0, :])
            vector.tensor_mul(
                wlsq[1][:, :], wlf_sb[:, 1, :], wlf_sb[:, 1, :]
            ).then_inc(s_sq, 2)
            # ||x||^2 for batch tiles 2/3: fp32 squares
            vector.wait_ge(s_xn[1], 16)
            for m in range(2):
                vector.tensor_mul(
                    xsq_scrD[:, m, :], xn_sb[1][:, m, :], xn_sb[1][:, m, :]
                )
            # broadcast -0.25*||w||^2 rows across the 4 m-slices
            vector.wait_ge(s_mm, 1)
            vector.tensor_copy(
                wsq_row4[:, :, :],
                bass.AP(tensor=ps_w, offset=0, ap=[[NLOC, 2], [0, MT], [1, NLOC]]),
            ).then_inc(s_brd)
            vector.drain()  # DVE RAW: reduces read xsq_scrD
            for m in range(2):
                inst = vector.tensor_reduce(
                    xsq_colD[:, m : m + 1], xsq_scrD[:, m, :],
                    axis=mybir.AxisListType.X, op=mybir.AluOpType.add,
                )
            inst.then_inc(s_colD)

        @block.tensor
        def _(tensor):
            # -0.25*||w||^2 reduce, two identical rows
            tensor.wait_ge(s_sq, 2)
            tensor.matmul(
                ps_w[:, :], lhsT=neg_q[:, :], rhs=wlsq[0][:, :],
                start=True, stop=False,
            )
            tensor.matmul(
                ps_w[:, :], lhsT=neg_q[:, :], rhs=wlsq[1][:, :],
                start=False, stop=True,
            ).then_inc(s_mm)  # = 1
            # main fp16 matmuls: one PSUM bank, single start on the first
            tensor.wait_ge(s_xt, 16)
            for k in range(KT):
                for m in range(MT):
                    tensor.matmul(
                        ps_all[:, m, :],
                        lhsT=xtf_sb[:, k, ts(m, P)],
                        rhs=wlf_sb[:, k, :],
                        start=(k == 0 and m == 0), stop=False,
                        skip_group_check=True,
                    )
            # fold -0.5*||w||^2 per m-slice (K=2: two -0.25 rows)
            tensor.wait_ge(s_brd, 1)
            for m in range(MT):
                tensor.matmul(
                    ps_all[:, m, :],
                    lhsT=ones_m[:, :],
                    rhs=wsq_row4[:, m, :],
                    start=False, stop=True, skip_group_check=True,
                ).then_inc(s_mm)  # = 2 + m

    nc.compile()
    return nc


def _get_nc():
    global _NC
    if _NC is None:
        _NC = _build()
    return _NC


def _make_in_maps(x: np.ndarray, weight: np.ndarray):
    x = np.ascontiguousarray(x.astype(np.float32, copy=False))
    xtf = np.ascontiguousarray(x.T.astype(np.float16))
    wf = weight.astype(np.float16)
    return [
        {
            "xtf": xtf,
            "xn": x,
            "wlf": np.ascontiguousarray(wf[:, c * NLOC : (c + 1) * NLOC]),
        }
        for c in range(NCORES)
    ]


def run(x: np.ndarray, weight: np.ndarray, trace: bool = False):
    """Returns (full_output, BassKernelResults)."""
    from concourse.bass_utils import run_bass_kernel_spmd

    nc = _get_nc()
    res = run_bass_kernel_spmd(
        nc, _make_in_maps(x, weight), core_ids=list(range(NCORES)), trace=trace
    )
    full = np.concatenate(
        [res.results[c]["out"] for c in range(NCORES)], axis=1
    )
    return full, res


def kernel(x: np.ndarray, weight: np.ndarray) -> np.ndarray:
    return run(x, weight)[0]



# revision 2
# speedup vs baseline: 1.2179x; 1.2179x over previous
"""Euclidean distance layer on 8 Trainium2 NeuronCores.

out[b, o] = || x[b, :] - weight[:, o] ||_2
x: [512, 256] f32, weight: [256, 1024] f32 -> out: [512, 1024] f32

Sharding: tensor-parallel over output features (8 x 128 columns per core).

Per core (all fp16 data, fp32 PSUM accumulation):
  psum[o, b] = x.w - 0.5*||x||^2      (features on PSUM partitions)
    - main: 2 matmuls, lhsT = w~[k,o] chunk, rhs = x~[k,b] (512-wide streams)
    - ||x||^2 fold: lhsT = const(-0.5) [128,128], rhs = x~^2 column sums
      (adds -0.5*||x_b||^2 to every feature row in one matmul)
  ||w||^2 via DVE square + ones-column matmul -> per-partition ACT bias
  out[o, b] = sqrt(-2*psum + ||w_o||^2) on ACT, fp16 out, 2 halves
  Sqrt ACT table warmed at block start (hides the ~1.3us table load
  behind the input DMA wait).
Host work is layout/dtype prep only (fp16 cast + transpose), output is
gathered as [o, b] fp16 per core and cast/transposed back on host.
"""

from contextlib import ExitStack

import numpy as np

B = 512      # batch
K = 256      # inputSize (contraction dim)
NOUT = 1024  # outputSize
NCORES = 8
NLOC = NOUT // NCORES  # 128 output features per core
P = 128                # partitions
KT = K // P            # 2 contraction chunks

_NC = None  # cached compiled Bass program (same SPMD program on all cores)


def _build():
    import concourse.bass as bass
    from concourse import bacc, mybir

    f32 = mybir.dt.float32
    f16 = mybir.dt.float16
    Sqrt = mybir.ActivationFunctionType.Sqrt

    nc = bacc.Bacc(
        "TRN2", target_bir_lowering=False, debug=False, num_devices=NCORES
    )

    xtf = nc.dram_tensor("xtf", [P, KT, B], f16, kind="ExternalInput")
    wlf = nc.dram_tensor("wlf", [P, KT, NLOC], f16, kind="ExternalInput")
    out = nc.dram_tensor("out", [NLOC, B], f16, kind="ExternalOutput")

    with ExitStack() as ctx:
        e = ctx.enter_context
        xtf_sb = e(nc.sbuf_tensor("xtfs", [P, KT, B], f16))
        wlf_sb = e(nc.sbuf_tensor("wlfs", [P, KT, NLOC], f16))
        wsq = e(nc.sbuf_tensor("wsq", [P, KT, NLOC], f16))
        xsq_a = e(nc.sbuf_tensor("xsq_a", [P, B], f16))
        xsq_b = e(nc.sbuf_tensor("xsq_b", [P, B], f16))
        xsq_s = e(nc.sbuf_tensor("xsq_s", [P, B], f16))
        ones_col = e(nc.sbuf_tensor("ones_col", [P, 1], f16))
        neghalf = e(nc.sbuf_tensor("neghalf", [P, P], f16))
        wsq_col = e(nc.sbuf_tensor("wsq_col", [P, 1], f32))
        out_sb = e(nc.sbuf_tensor("out_sb", [P, B], f16))
        warm = e(nc.sbuf_tensor("warm", [1, 1], f32))

        ps_main = e(nc.psum_tensor("ps_main", [P, B], f32))
        ps_wcol = e(nc.psum_tensor("ps_wcol", [P, 1], f32))

        s_x = [e(nc.semaphore(f"s_x{k}")) for k in range(KT)]
        s_w = e(nc.semaphore("s_w"))
        s_wsq = e(nc.semaphore("s_wsq"))    # 1 = w squares in SBUF
        s_xsq = e(nc.semaphore("s_xsq"))    # 1 = x column sums in SBUF
        s_wc = e(nc.semaphore("s_wc"))      # 1 = ps_wcol reduced
        s_wcb = e(nc.semaphore("s_wcb"))    # 1 = wsq_col bias in SBUF
        s_main = e(nc.semaphore("s_main"))  # 1 = ps_main complete
        s_sq = e(nc.semaphore("s_sq"))      # h+1 = sqrt half h in out_sb
        s_od0 = e(nc.semaphore("s_od0"))    # 16 = output DMA half 0 landed
        s_od1 = e(nc.semaphore("s_od1"))    # 16 = output DMA half 1 landed

        block = e(nc.Block())

        @block.sync
        def _(sync):
            for k in range(KT):
                sync.dma_start(
                    out=xtf_sb[:, k, :], in_=xtf[:, k, :]
                ).then_inc(s_x[k], 16)
            sync.wait_ge(s_sq, 1)
            sync.dma_start(
                out=out[:, 0 : B // 2], in_=out_sb[:, 0 : B // 2]
            ).then_inc(s_od0, 16)
            sync.wait_ge(s_od0, 16)

        @block.gpsimd
        def _(gpsimd):
            gpsimd.dma_start(out=wlf_sb[:, :, :], in_=wlf[:, :, :]).then_inc(
                s_w, 16
            )
            gpsimd.wait_ge(s_w, 16)

        @block.vector
        def _(vector):
            vector.memset(ones_col[:, :], 1.0)
            vector.memset(neghalf[:, :], -0.5)
            vector.wait_ge(s_w, 16)
            vector.tensor_mul(
                wsq[:, :, :], wlf_sb[:, :, :], wlf_sb[:, :, :]
            ).then_inc(s_wsq)
            vector.wait_ge(s_x[0], 16)
            vector.tensor_mul(xsq_a[:, :], xtf_sb[:, 0, :], xtf_sb[:, 0, :])
            vector.wait_ge(s_x[1], 16)
            vector.tensor_mul(xsq_b[:, :], xtf_sb[:, 1, :], xtf_sb[:, 1, :])
            vector.drain()  # DVE RAW: the add reads xsq_a/xsq_b
            vector.tensor_add(xsq_s[:, :], xsq_a[:, :], xsq_b[:, :]).then_inc(
                s_xsq
            )
            vector.wait_ge(s_wc, 1)
            vector.tensor_copy(wsq_col[:, :], ps_wcol[:, :]).then_inc(s_wcb)

        @block.tensor
        def _(tensor):
            # ||w||^2 column: ones-vector reduce of the fp16 squares
            tensor.wait_ge(s_wsq, 1)
            tensor.matmul(
                ps_wcol[:, :], lhsT=wsq[:, 0, :], rhs=ones_col[:, :],
                start=True, stop=False,
            )
            tensor.matmul(
                ps_wcol[:, :], lhsT=wsq[:, 1, :], rhs=ones_col[:, :],
                start=False, stop=True,
            ).then_inc(s_wc)
            # main: psum[o, b] = sum_k w~[k,o] x~[k,b]
            for k in range(KT):
                tensor.wait_ge(s_x[k], 16)
                tensor.matmul(
                    ps_main[:, :], lhsT=wlf_sb[:, k, :], rhs=xtf_sb[:, k, :],
                    start=(k == 0), stop=False,
                    skip_group_check=(k != 0),
                )
            # fold -0.5*||x_b||^2 into every feature row
            tensor.wait_ge(s_xsq, 1)
            tensor.matmul(
                ps_main[:, :], lhsT=neghalf[:, :], rhs=xsq_s[:, :],
                start=False, stop=True, skip_group_check=True,
            ).then_inc(s_main)

        @block.scalar
        def _(scalar):
            # warm the Sqrt ACT table while the input DMAs are in flight
            scalar.activation(warm[:, :], warm[:, :], Sqrt)
            scalar.wait_ge(s_wcb, 1)
            scalar.wait_ge(s_main, 1)
            for h in range(2):
                sl = slice(h * (B // 2), (h + 1) * (B // 2))
                scalar.activation(
                    out_sb[:, sl], ps_main[:, sl], Sqrt,
                    bias=wsq_col[:, 0:1], scale=-2.0,
                ).then_inc(s_sq)
            # sem fires at retire, so out_sb half 1 is written by now
            scalar.wait_ge(s_sq, 2)
            scalar.dma_start(
                out=out[:, B // 2 : B], in_=out_sb[:, B // 2 : B]
            ).then_inc(s_od1, 16)
            scalar.wait_ge(s_od1, 16)

    nc.compile()
    return nc


def _get_nc():
    global _NC
    if _NC is None:
        _NC = _build()
    return _NC


def _make_in_maps(x: np.ndarray, weight: np.ndarray):
    # xtf[p, kc, b] = x[b, kc*128 + p], fp16
    xtf = np.ascontiguousarray(
        x.astype(np.float16).T.reshape(KT, P, B).transpose(1, 0, 2)
    )
    wf = weight.astype(np.float16)
    return [
        {
            "xtf": xtf,
            # wlf[p, kc, o] = weight[kc*128 + p, c*128 + o], fp16
            "wlf": np.ascontiguousarray(
                wf[:, c * NLOC : (c + 1) * NLOC]
                .reshape(KT, P, NLOC)
                .transpose(1, 0, 2)
            ),
        }
        for c in range(NCORES)
    ]


def run(x: np.ndarray, weight: np.ndarray, trace: bool = False):
    """Returns (full_output, BassKernelResults)."""
    from concourse.bass_utils import run_bass_kernel_spmd

    nc = _get_nc()
    res = run_bass_kernel_spmd(
        nc, _make_in_maps(x, weight), core_ids=list(range(NCORES)), trace=trace
    )
    full = np.concatenate(
        [res.results[c]["out"].astype(np.float32).T for c in range(NCORES)],
        axis=1,
    )
    return full, res


def kernel(x: np.ndarray, weight: np.ndarray) -> np.ndarray:
    return run(x, weight)[0]
